# revision 9
# baseline (speedup 1.0000x reference)
"""Gaussian falloff vortex-velocity kernel for Trainium2 (8 NeuronCores).

Math: out[b,p,:] = sum_n tau_n * exp(-r2/sig_n^2) / sqrt(r2) * (d2, -d1)
with d1 = py - y_n, d2 = px - x_n, r2 = d1^2 + d2^2.

Device pipeline (per 512-point tile x 128-particle block):
  1. rho = m_n * (|p'|^2 - 2 p'.loc'_n + |loc'_n|^2 + eps_T)   [PE matmul,
     18-row bf16 hi/mid/lo split contraction -> fp32 PSUM, exact to ~eps_T/8]
  2. lt = Ln(rho)                                               [ACT, no scale]
  3. wt = 2*c_n*rho + lt            (c_n = (1/sig^2)/m_n)       [DVE stt]
  4. g~ = Exp(-0.5 * wt) -> bf16                                [ACT, const scale]
     (= exp(-r2/sig^2) * rsqrt(r2+eps) / sqrt(m_n))
  5. S[0:3] += w3_n . g~   (w3 = tau*sqrt(m) * {1, x', y'})     [PE matmul]
  u = px'*S0 - S1 ; v = S2 - py'*S0.

m_n is a power of two (= 2^round(log2(1/sig^2))): scaling bf16 weights by it is
exact, so the split-matmul cancellation survives.  p', loc' are per-tile
translated coordinates (points kd-sorted into spatially tight 512-point tiles),
which makes the absolute r2 error scale with the tile radius; eps_T is sized to
the tile's own error bound.

Sharding: each core takes 16 of the 128 kd-tiles per batch (all particles).
"""

import sys

import numpy as np

B, H, W, N = 2, 256, 256, 512
P = H * W                  # 65536 points per batch
NCORES = 8
NTILES = 128               # kd tiles per batch, 512 points each
TP = 512                   # points per tile
TPC = NTILES // NCORES     # 16 tiles per core per batch
PPB = TPC * TP             # 8192 points per batch per core
NK = N // 128              # 4 particle blocks
NR = 18                    # matmul1 contraction rows
EPS_COEF = 1.5e-4

_cache = {}


def _bass_modules():
    if "/opt/trn_rl_repo" not in sys.path:
        sys.path.insert(0, "/opt/trn_rl_repo")
    import concourse.bass as bass
    import concourse.mybir as mybir
    import concourse.tile as tile
    from concourse import bacc
    from concourse.bass_utils import run_bass_kernel_spmd

    return bass, mybir, tile, run_bass_kernel_spmd, bacc


def _bf16(x):
    x = np.asarray(x, dtype=np.float32)
    u = x.view(np.uint32)
    r = (u + 0x7FFF + ((u >> 16) & 1)) & np.uint32(0xFFFF0000)
    return r.view(np.float32)


def _split3(x):
    h = _bf16(x)
    m = _bf16(x - h)
    l = _bf16(x - h - m)
    return h, m, l


def _kd_sort(py, px, ntiles):
    """Permutation putting points into ntiles spatially tight equal chunks."""
    idx = np.arange(py.shape[0])

    def rec(ids, nt):
        if nt == 1:
            return [ids]
        ay, ax = py[ids], px[ids]
        coords = ay if (ay.max() - ay.min()) >= (ax.max() - ax.min()) else ax
        half = len(ids) // 2
        part = np.argpartition(coords, half)
        return rec(ids[part[:half]], nt // 2) + rec(ids[part[half:]], nt // 2)

    return np.concatenate(rec(idx, ntiles))


def _build_nc():
    bass, mybir, tile, _, bacc = _bass_modules()
    f32 = mybir.dt.float32
    bf16 = mybir.dt.bfloat16
    AF = mybir.ActivationFunctionType
    ALU = mybir.AluOpType

    nc = bacc.Bacc(None)

    # Force Ln and Exp onto the single table set that holds both
    # (natural_log_exp_and_others); otherwise the set chooser alternates
    # between the ln-only and exp-only sets, paying a ~1.3us ACT_TABLE_LOAD
    # per activation. get_activation_tables is functools.cache'd, so editing
    # the returned dict in place (names/indices preserved) steers the pass.
    from concourse.hw_specs import get_activation_tables

    tabs = get_activation_tables(nc.m.arch)
    both = tabs.get("natural_log_exp_and_others")
    if both is not None and AF.Exp in both and AF.Ln in both:
        for nm, funcs in tabs.items():
            if nm != "natural_log_exp_and_others":
                funcs.discard(AF.Exp)
                funcs.discard(AF.Ln)

    rr_d = nc.declare_dram_parameter("rr", [B, TPC, NR, TP], bf16, isOutput=False)
    w1_d = nc.declare_dram_parameter("w1", [B, TPC, NR, NK * 128], bf16, isOutput=False)
    w3_d = nc.declare_dram_parameter("w3", [B, TPC, 128, NK * 3], bf16, isOutput=False)
    c2_d = nc.declare_dram_parameter("c2", [128, B * NK], f32, isOutput=False)
    ptsf_d = nc.declare_dram_parameter("ptsf", [B, 2, 128, PPB // 128], f32, isOutput=False)
    out_d = nc.declare_dram_parameter("out", [B, 2, 128, PPB // 128], f32, isOutput=True)

    with tile.TileContext(nc) as tc:
        with (
            tc.tile_pool(name="const", bufs=1) as cpool,
            tc.tile_pool(name="inp", bufs=3) as inp,
            tc.tile_pool(name="work", bufs=2) as work,
            tc.tile_pool(name="psq", bufs=3, space=bass.MemorySpace.PSUM) as psq,
            tc.tile_pool(name="psacc", bufs=2, space=bass.MemorySpace.PSUM) as psacc,
            tc.tile_pool(name="sout", bufs=3) as soutp,
            tc.tile_pool(name="fin", bufs=2) as fin,
            tc.tile_pool(name="dscratch", bufs=1, space="DRAM") as dpool,
        ):
            c2s = cpool.tile([128, B * NK], f32)
            nc.sync.dma_start(c2s[:], c2_d[:])
            scratch = dpool.tile([B, 3, PPB], f32)

            # PE clock warmup: the steady-state PE duty cycle (~50%, ~2us
            # bursts) never sustains a full HAM activity window, so without
            # this the PE runs at 1.2 GHz (K=4/8) for the whole kernel and
            # MM1 gates the scalar engine. ~16 back-to-back matmuls give the
            # ~3.4us of contiguous PE-busy needed to flip to 2.4 GHz; the
            # steady-state gaps are too short to ever re-throttle.
            rrw = cpool.tile([NR, TP], bf16)
            nc.sync.dma_start(rrw[:], rr_d[0, 0])
            w1w = cpool.tile([NR, 128], bf16)
            nc.sync.dma_start(w1w[:], w1_d[0, 0, :, 0:128])
            qw = psq.tile([128, 2 * TP], f32, tag="q")
            for _ in range(16):
                nc.tensor.matmul(qw[:, 0:TP], w1w[:], rrw[:], start=True, stop=True)

            # 3-stage software pipeline over the 32 (b, T) tiles so the
            # per-engine instruction streams never head-of-line block:
            #   iter i: [A] DMA+MM1+Ln+stt for tile i   (PE, ACT, DVE)
            #           [B] Exp for tile i-1            (ACT)
            #           [C] MM2+copy+store for tile i-2 (PE, DVE)
            # This keeps MM1(i) ahead of MM2(i-2) in the PE queue and lets
            # ACT run Ln(i) while DVE finishes tile i-1's stt.
            NTL = B * TPC
            wts, gs, w3s = {}, {}, {}
            for i in range(NTL + 2):
                if i < NTL:
                    b, T = divmod(i, TPC)
                    rr = inp.tile([NR, TP], bf16, tag="rr")
                    nc.sync.dma_start(rr[:], rr_d[b, T])
                    w1 = inp.tile([NR, NK * 128], bf16, tag="w1")
                    nc.sync.dma_start(w1[:], w1_d[b, T])
                    w3 = inp.tile([128, NK * 3], bf16, tag="w3")
                    nc.sync.dma_start(w3[:], w3_d[b, T])
                    w3s[i] = w3

                    wt = work.tile([128, NK * TP], f32, tag="wt")
                    wts[i] = wt
                    for h in range(2):
                        q = psq.tile([128, 2 * TP], f32, tag="q")
                        for k2 in range(2):
                            k = 2 * h + k2
                            nc.tensor.matmul(
                                q[:, k2 * TP : (k2 + 1) * TP],
                                w1[:, k * 128 : (k + 1) * 128],
                                rr[:],
                                start=True,
                                stop=True,
                            )
                        lt = work.tile([128, 2 * TP], f32, tag="lt")
                        nc.scalar.activation(lt[:], q[:], AF.Ln)
                        for k2 in range(2):
                            k = 2 * h + k2
                            col = b * NK + k
                            nc.vector.scalar_tensor_tensor(
                                wt[:, k * TP : (k + 1) * TP],
                                q[:, k2 * TP : (k2 + 1) * TP],
                                c2s[:, col : col + 1],
                                lt[:, k2 * TP : (k2 + 1) * TP],
                                ALU.mult,
                                ALU.add,
                            )
                if 1 <= i <= NTL:
                    j = i - 1
                    g = work.tile([128, NK * TP], bf16, tag="g")
                    gs[j] = g
                    nc.scalar.activation(g[:], wts.pop(j)[:], AF.Exp, scale=-0.5)
                if i >= 2:
                    j = i - 2
                    b2, T2 = divmod(j, TPC)
                    g = gs.pop(j)
                    w3 = w3s.pop(j)
                    sacc = psacc.tile([3, TP], f32, tag="sacc")
                    for k in range(NK):
                        nc.tensor.matmul(
                            sacc[:],
                            w3[:, k * 3 : (k + 1) * 3],
                            g[:, k * TP : (k + 1) * TP],
                            start=(k == 0),
                            stop=(k == NK - 1),
                        )
                    srow = soutp.tile([3, TP], f32, tag="srow")
                    nc.vector.tensor_copy(srow[:], sacc[:])
                    nc.sync.dma_start(scratch[b2, :, T2 * TP : (T2 + 1) * TP], srow[:])

            srs = scratch[:].rearrange("b three (p f) -> b three p f", p=128)
            for b in range(B):
                s0 = fin.tile([128, PPB // 128], f32, tag="s0")
                nc.sync.dma_start(s0[:], srs[b, 0])
                s1 = fin.tile([128, PPB // 128], f32, tag="s1")
                nc.sync.dma_start(s1[:], srs[b, 1])
                s2 = fin.tile([128, PPB // 128], f32, tag="s2")
                nc.sync.dma_start(s2[:], srs[b, 2])
                pyf = fin.tile([128, PPB // 128], f32, tag="pyf")
                nc.sync.dma_start(pyf[:], ptsf_d[b, 0])
                pxf = fin.tile([128, PPB // 128], f32, tag="pxf")
                nc.sync.dma_start(pxf[:], ptsf_d[b, 1])
                tu = fin.tile([128, PPB // 128], f32, tag="tu")
                nc.vector.tensor_mul(tu[:], pxf[:], s0[:])
                u = fin.tile([128, PPB // 128], f32, tag="u")
                nc.vector.tensor_sub(u[:], tu[:], s1[:])
                tv = fin.tile([128, PPB // 128], f32, tag="tv")
                nc.vector.tensor_mul(tv[:], pyf[:], s0[:])
                v = fin.tile([128, PPB // 128], f32, tag="v")
                nc.vector.tensor_sub(v[:], s2[:], tv[:])
                nc.sync.dma_start(out_d[b, 0], u[:])
                nc.sync.dma_start(out_d[b, 1], v[:])
    nc.compile()
    return nc


def _prep_inputs(vortex_feature, points):
    import ml_dtypes

    mbf = ml_dtypes.bfloat16
    vf = np.asarray(vortex_feature, dtype=np.float32)
    pts_full = np.asarray(points, dtype=np.float32)

    perms = []
    # per-core staging arrays
    rr_all = np.zeros((NCORES, B, TPC, NR, TP), np.float32)
    w1_all = np.zeros((NCORES, B, TPC, NR, NK * 128), np.float32)
    w3_all = np.zeros((NCORES, B, TPC, 128, NK * 3), np.float32)
    ptsf_all = np.zeros((NCORES, B, 2, PPB), np.float32)
    c2 = np.zeros((128, B * NK), np.float32)

    for b in range(B):
        y, x, tau, sig = vf[b, :, 0], vf[b, :, 1], vf[b, :, 2], vf[b, :, 3]
        nn = 1.0 / (sig * sig)
        m = np.exp2(np.round(np.log2(nn))).astype(np.float32)
        c = (nn / m).astype(np.float32)
        sm = np.sqrt(m).astype(np.float32)
        c2[:, b * NK : (b + 1) * NK] = (2.0 * c).reshape(NK, 128).T

        pyA = pts_full[b, :, :, 0].reshape(-1)
        pxA = pts_full[b, :, :, 1].reshape(-1)
        perm = _kd_sort(pyA, pxA, NTILES)
        perms.append(perm)
        pys = pyA[perm].reshape(NTILES, TP)
        pxs = pxA[perm].reshape(NTILES, TP)

        # tile centers and translated coords, vectorized over tiles
        sy = (pys.min(1) + pys.max(1)) * 0.5          # [NTILES]
        sx = (pxs.min(1) + pxs.max(1)) * 0.5
        pyt = (pys - sy[:, None]).astype(np.float32)   # [NTILES, TP]
        pxt = (pxs - sx[:, None]).astype(np.float32)
        rT2 = (pyt**2 + pxt**2).max(1)                 # [NTILES]
        epsT = np.maximum(EPS_COEF * np.maximum(rT2, 0.01), 1e-6).astype(np.float32)

        yt = (y[None, :] - sy[:, None]).astype(np.float32)   # [NTILES, N]
        xt = (x[None, :] - sx[:, None]).astype(np.float32)
        l2e = (yt.astype(np.float64) ** 2 + xt.astype(np.float64) ** 2
               + epsT[:, None]).astype(np.float32)

        yh, ym, yl = _split3(yt)
        xh, xm, xl = _split3(xt)
        lh, lm, ll = _split3(l2e)
        mb = np.broadcast_to(m[None, :], (NTILES, N))
        W1 = _bf16(np.stack([
            -2 * m * yh, -2 * m * yh, -2 * m * ym, -2 * m * ym, -2 * m * yh, -2 * m * yl,
            -2 * m * xh, -2 * m * xh, -2 * m * xm, -2 * m * xm, -2 * m * xh, -2 * m * xl,
            mb, mb, mb,
            m * lh, m * lm, m * ll,
        ], axis=1).astype(np.float32))                 # [NTILES, NR, N]

        pyh, pym, pyl = _split3(pyt)
        pxh, pxm, pxl = _split3(pxt)
        p2 = (pyt.astype(np.float64) ** 2 + pxt.astype(np.float64) ** 2).astype(np.float32)
        p2h, p2m, p2l = _split3(p2)
        ones = np.ones((NTILES, TP), np.float32)
        RR = np.stack([
            pyh, pym, pyh, pym, pyl, pyh,
            pxh, pxm, pxh, pxm, pxl, pxh,
            p2h, p2m, p2l,
            ones, ones, ones,
        ], axis=1).astype(np.float32)                  # [NTILES, NR, TP]

        tsm = (tau * sm).astype(np.float32)
        W3 = _bf16(np.stack([
            np.broadcast_to(tsm[None, :], (NTILES, N)),
            xt * tsm[None, :],
            yt * tsm[None, :],
        ], axis=1).astype(np.float32))                 # [NTILES, 3, N]

        for i in range(NCORES):
            tl = slice(i * TPC, (i + 1) * TPC)
            rr_all[i, b] = RR[tl]
            # w1: [NR, NK*128] with col = k*128 + j
            w1_all[i, b] = W1[tl].reshape(TPC, NR, N)
            # w3: [128, NK*3] with col = k*3 + r ; element [j, k*3+r] = W3[T, r, k*128+j]
            w3c = W3[tl].reshape(TPC, 3, NK, 128).transpose(0, 3, 2, 1)  # [TPC,128,NK,3]
            w3_all[i, b] = w3c.reshape(TPC, 128, NK * 3)
            ptsf_all[i, b, 0] = pyt[tl].reshape(-1)
            ptsf_all[i, b, 1] = pxt[tl].reshape(-1)

    in_maps = []
    for i in range(NCORES):
        in_maps.append({
            "rr": rr_all[i].astype(mbf),
            "w1": w1_all[i].astype(mbf),
            "w3": w3_all[i].astype(mbf),
            "c2": c2,
            "ptsf": np.ascontiguousarray(ptsf_all[i].reshape(B, 2, 128, PPB // 128)),
        })
    return in_maps, perms


def _assemble(results, perms):
    out = np.zeros((B, P, 2), dtype=np.float32)
    for b in range(B):
        vals = np.concatenate(
            [np.asarray(results[i]["out"])[b].reshape(2, PPB) for i in range(NCORES)],
            axis=1,
        )  # [2, P] in sorted order
        out[b, perms[b], 0] = vals[0]
        out[b, perms[b], 1] = vals[1]
    return out.reshape(B, H, W, 2)


def _run(vortex_feature, points, trace=False):
    _, _, _, run_bass_kernel_spmd, _b = _bass_modules()
    if "nc" not in _cache:
        _cache["nc"] = _build_nc()
    in_maps, perms = _prep_inputs(vortex_feature, points)
    res = run_bass_kernel_spmd(
        _cache["nc"], in_maps, list(range(NCORES)), trace=trace
    )
    return _assemble(res.results, perms), res


def kernel(vortex_feature, points):
    out, _ = _run(vortex_feature, points, trace=False)
    return out


# revision 12
# speedup vs baseline: 1.1155x; 1.1155x over previous
"""Gaussian falloff vortex-velocity kernel for Trainium2 (8 NeuronCores).

Math: out[b,p,:] = sum_n tau_n * exp(-r2/sig_n^2) / sqrt(r2) * (d2, -d1)
with d1 = py - y_n, d2 = px - x_n, r2 = d1^2 + d2^2.

Device pipeline (per 512-point tile x 128-particle block):
  1. rho = m_n * (|p'|^2 - 2 p'.loc'_n + |loc'_n|^2 + eps_T)   [PE matmul,
     18-row bf16 hi/mid/lo split contraction -> fp32 PSUM, exact to ~eps_T/8]
  2. lt = Ln(rho)                                               [ACT, no scale]
  3. wt = 2*c_n*rho + lt            (c_n = (1/sig^2)/m_n)       [DVE stt]
  4. g~ = Exp(-0.5 * wt) -> bf16                                [ACT, const scale]
     (= exp(-r2/sig^2) * rsqrt(r2+eps) / sqrt(m_n))
  5. S[0:3] += w3_n . g~   (w3 = tau*sqrt(m) * {1, x', y'})     [PE matmul]
  u = px'*S0 - S1 ; v = S2 - py'*S0.

m_n is a power of two (= 2^round(log2(1/sig^2))): scaling bf16 weights by it is
exact, so the split-matmul cancellation survives.  p', loc' are per-tile
translated coordinates (points kd-sorted into spatially tight 512-point tiles),
which makes the absolute r2 error scale with the tile radius; eps_T is sized to
the tile's own error bound.

Sharding: each core takes 16 of the 128 kd-tiles per batch (all particles).
"""

import sys

import numpy as np

B, H, W, N = 2, 256, 256, 512
P = H * W                  # 65536 points per batch
NCORES = 8
NTILES = 128               # kd tiles per batch, 512 points each
TP = 512                   # points per tile
TPC = NTILES // NCORES     # 16 tiles per core per batch
PPB = TPC * TP             # 8192 points per batch per core
NK = N // 128              # 4 particle blocks
NR = 18                    # matmul1 contraction rows
EPS_COEF = 1.5e-4

_cache = {}


def _bass_modules():
    if "/opt/trn_rl_repo" not in sys.path:
        sys.path.insert(0, "/opt/trn_rl_repo")
    import concourse.bass as bass
    import concourse.mybir as mybir
    import concourse.tile as tile
    from concourse import bacc
    from concourse.bass_utils import run_bass_kernel_spmd

    return bass, mybir, tile, run_bass_kernel_spmd, bacc


def _bf16(x):
    x = np.asarray(x, dtype=np.float32)
    u = x.view(np.uint32)
    r = (u + 0x7FFF + ((u >> 16) & 1)) & np.uint32(0xFFFF0000)
    return r.view(np.float32)


def _split3(x):
    h = _bf16(x)
    m = _bf16(x - h)
    l = _bf16(x - h - m)
    return h, m, l


def _kd_sort(py, px, ntiles):
    """Permutation putting points into ntiles spatially tight equal chunks."""
    idx = np.arange(py.shape[0])

    def rec(ids, nt):
        if nt == 1:
            return [ids]
        ay, ax = py[ids], px[ids]
        coords = ay if (ay.max() - ay.min()) >= (ax.max() - ax.min()) else ax
        half = len(ids) // 2
        part = np.argpartition(coords, half)
        return rec(ids[part[:half]], nt // 2) + rec(ids[part[half:]], nt // 2)

    return np.concatenate(rec(idx, ntiles))


def _build_nc():
    bass, mybir, tile, _, bacc = _bass_modules()
    f32 = mybir.dt.float32
    bf16 = mybir.dt.bfloat16
    AF = mybir.ActivationFunctionType
    ALU = mybir.AluOpType

    nc = bacc.Bacc(None)

    # Force Ln and Exp onto the single table set that holds both
    # (natural_log_exp_and_others); otherwise the set chooser alternates
    # between the ln-only and exp-only sets, paying a ~1.3us ACT_TABLE_LOAD
    # per activation. get_activation_tables is functools.cache'd, so editing
    # the returned dict in place (names/indices preserved) steers the pass.
    from concourse.hw_specs import get_activation_tables

    tabs = get_activation_tables(nc.m.arch)
    both = tabs.get("natural_log_exp_and_others")
    if both is not None and AF.Exp in both and AF.Ln in both:
        for nm, funcs in tabs.items():
            if nm != "natural_log_exp_and_others":
                funcs.discard(AF.Exp)
                funcs.discard(AF.Ln)

    rr_d = nc.declare_dram_parameter("rr", [B, TPC, 128, TP], bf16, isOutput=False)
    w1_d = nc.declare_dram_parameter("w1", [B, TPC, 128, 128], bf16, isOutput=False)
    w3_d = nc.declare_dram_parameter("w3", [B, TPC, 128, NK * 3], bf16, isOutput=False)
    c2_d = nc.declare_dram_parameter("c2", [128, B * NK], f32, isOutput=False)
    ptsf_d = nc.declare_dram_parameter("ptsf", [B, 2, 128, PPB // 128], f32, isOutput=False)
    out_d = nc.declare_dram_parameter("out", [B, 2, 128, PPB // 128], f32, isOutput=True)

    with tile.TileContext(nc) as tc:
        with (
            tc.tile_pool(name="const", bufs=1) as cpool,
            tc.tile_pool(name="inp", bufs=3) as inp,
            tc.tile_pool(name="work", bufs=2) as work,
            tc.tile_pool(name="psq", bufs=3, space=bass.MemorySpace.PSUM) as psq,
            tc.tile_pool(name="psacc", bufs=2, space=bass.MemorySpace.PSUM) as psacc,
            tc.tile_pool(name="sout", bufs=3) as soutp,
            tc.tile_pool(name="fin", bufs=2) as fin,
            tc.tile_pool(name="dscratch", bufs=1, space="DRAM") as dpool,
        ):
            c2s = cpool.tile([128, B * NK], f32)
            nc.sync.dma_start(c2s[:], c2_d[:])
            scratch = dpool.tile([B, 3, PPB], f32)

            # 3-stage software pipeline over the 32 (b, T) tiles so the
            # per-engine instruction streams never head-of-line block:
            #   iter i: [A] DMA+MM1+Ln+stt for tile i   (PE, ACT, DVE)
            #           [B] Exp for tile i-1            (ACT)
            #           [C] MM2+copy+store for tile i-2 (PE, DVE)
            # This keeps MM1(i) ahead of MM2(i-2) in the PE queue and lets
            # ACT run Ln(i) while DVE finishes tile i-1's stt.
            NTL = B * TPC
            wts, gs, w3s = {}, {}, {}
            for i in range(NTL + 2):
                if i < NTL:
                    b, T = divmod(i, TPC)
                    rr = inp.tile([128, TP], bf16, tag="rr")
                    nc.sync.dma_start(rr[:], rr_d[b, T])
                    w1 = inp.tile([128, 128], bf16, tag="w1")
                    nc.sync.dma_start(w1[:], w1_d[b, T])
                    w3 = inp.tile([128, NK * 3], bf16, tag="w3")
                    nc.sync.dma_start(w3[:], w3_d[b, T])
                    w3s[i] = w3

                    wt = work.tile([128, NK * TP], f32, tag="wt")
                    wts[i] = wt
                    for h in range(2):
                        q = psq.tile([128, 2 * TP], f32, tag="q")
                        for k2 in range(2):
                            k = 2 * h + k2
                            nc.tensor.matmul(
                                q[:, k2 * TP : (k2 + 1) * TP],
                                w1[32 * k : 32 * k + NR, :],
                                rr[32 * k : 32 * k + NR, :],
                                start=True,
                                stop=True,
                                tile_position=(32 * k, 0),
                            )
                        lt = work.tile([128, 2 * TP], f32, tag="lt")
                        nc.scalar.activation(lt[:], q[:], AF.Ln)
                        for k2 in range(2):
                            k = 2 * h + k2
                            col = b * NK + k
                            nc.vector.scalar_tensor_tensor(
                                wt[:, k * TP : (k + 1) * TP],
                                q[:, k2 * TP : (k2 + 1) * TP],
                                c2s[:, col : col + 1],
                                lt[:, k2 * TP : (k2 + 1) * TP],
                                ALU.mult,
                                ALU.add,
                            )
                if 1 <= i <= NTL:
                    j = i - 1
                    g = work.tile([128, NK * TP], bf16, tag="g")
                    gs[j] = g
                    nc.scalar.activation(g[:], wts.pop(j)[:], AF.Exp, scale=-0.5)
                if i >= 2:
                    j = i - 2
                    b2, T2 = divmod(j, TPC)
                    g = gs.pop(j)
                    w3 = w3s.pop(j)
                    sacc = psacc.tile([3, TP], f32, tag="sacc")
                    for k in range(NK):
                        nc.tensor.matmul(
                            sacc[:],
                            w3[:, k * 3 : (k + 1) * 3],
                            g[:, k * TP : (k + 1) * TP],
                            start=(k == 0),
                            stop=(k == NK - 1),
                        )
                    srow = soutp.tile([3, TP], f32, tag="srow")
                    nc.vector.tensor_copy(srow[:], sacc[:])
                    nc.sync.dma_start(scratch[b2, :, T2 * TP : (T2 + 1) * TP], srow[:])

            srs = scratch[:].rearrange("b three (p f) -> b three p f", p=128)
            for b in range(B):
                s0 = fin.tile([128, PPB // 128], f32, tag="s0")
                nc.sync.dma_start(s0[:], srs[b, 0])
                s1 = fin.tile([128, PPB // 128], f32, tag="s1")
                nc.sync.dma_start(s1[:], srs[b, 1])
                s2 = fin.tile([128, PPB // 128], f32, tag="s2")
                nc.sync.dma_start(s2[:], srs[b, 2])
                pyf = fin.tile([128, PPB // 128], f32, tag="pyf")
                nc.sync.dma_start(pyf[:], ptsf_d[b, 0])
                pxf = fin.tile([128, PPB // 128], f32, tag="pxf")
                nc.sync.dma_start(pxf[:], ptsf_d[b, 1])
                tu = fin.tile([128, PPB // 128], f32, tag="tu")
                nc.vector.tensor_mul(tu[:], pxf[:], s0[:])
                u = fin.tile([128, PPB // 128], f32, tag="u")
                nc.vector.tensor_sub(u[:], tu[:], s1[:])
                tv = fin.tile([128, PPB // 128], f32, tag="tv")
                nc.vector.tensor_mul(tv[:], pyf[:], s0[:])
                v = fin.tile([128, PPB // 128], f32, tag="v")
                nc.vector.tensor_sub(v[:], s2[:], tv[:])
                nc.sync.dma_start(out_d[b, 0], u[:])
                nc.sync.dma_start(out_d[b, 1], v[:])
    nc.compile()
    return nc


def _prep_inputs(vortex_feature, points):
    import ml_dtypes

    mbf = ml_dtypes.bfloat16
    vf = np.asarray(vortex_feature, dtype=np.float32)
    pts_full = np.asarray(points, dtype=np.float32)

    perms = []
    # per-core staging arrays
    rr_all = np.zeros((NCORES, B, TPC, 128, TP), np.float32)
    w1_all = np.zeros((NCORES, B, TPC, 128, 128), np.float32)
    w3_all = np.zeros((NCORES, B, TPC, 128, NK * 3), np.float32)
    ptsf_all = np.zeros((NCORES, B, 2, PPB), np.float32)
    c2 = np.zeros((128, B * NK), np.float32)

    for b in range(B):
        y, x, tau, sig = vf[b, :, 0], vf[b, :, 1], vf[b, :, 2], vf[b, :, 3]
        nn = 1.0 / (sig * sig)
        m = np.exp2(np.round(np.log2(nn))).astype(np.float32)
        c = (nn / m).astype(np.float32)
        sm = np.sqrt(m).astype(np.float32)
        c2[:, b * NK : (b + 1) * NK] = (2.0 * c).reshape(NK, 128).T

        pyA = pts_full[b, :, :, 0].reshape(-1)
        pxA = pts_full[b, :, :, 1].reshape(-1)
        perm = _kd_sort(pyA, pxA, NTILES)
        perms.append(perm)
        pys = pyA[perm].reshape(NTILES, TP)
        pxs = pxA[perm].reshape(NTILES, TP)

        # tile centers and translated coords, vectorized over tiles
        sy = (pys.min(1) + pys.max(1)) * 0.5          # [NTILES]
        sx = (pxs.min(1) + pxs.max(1)) * 0.5
        pyt = (pys - sy[:, None]).astype(np.float32)   # [NTILES, TP]
        pxt = (pxs - sx[:, None]).astype(np.float32)
        rT2 = (pyt**2 + pxt**2).max(1)                 # [NTILES]
        epsT = np.maximum(EPS_COEF * np.maximum(rT2, 0.01), 1e-6).astype(np.float32)

        yt = (y[None, :] - sy[:, None]).astype(np.float32)   # [NTILES, N]
        xt = (x[None, :] - sx[:, None]).astype(np.float32)
        l2e = (yt.astype(np.float64) ** 2 + xt.astype(np.float64) ** 2
               + epsT[:, None]).astype(np.float32)

        yh, ym, yl = _split3(yt)
        xh, xm, xl = _split3(xt)
        lh, lm, ll = _split3(l2e)
        mb = np.broadcast_to(m[None, :], (NTILES, N))
        W1 = _bf16(np.stack([
            -2 * m * yh, -2 * m * yh, -2 * m * ym, -2 * m * ym, -2 * m * yh, -2 * m * yl,
            -2 * m * xh, -2 * m * xh, -2 * m * xm, -2 * m * xm, -2 * m * xh, -2 * m * xl,
            mb, mb, mb,
            m * lh, m * lm, m * ll,
        ], axis=1).astype(np.float32))                 # [NTILES, NR, N]

        pyh, pym, pyl = _split3(pyt)
        pxh, pxm, pxl = _split3(pxt)
        p2 = (pyt.astype(np.float64) ** 2 + pxt.astype(np.float64) ** 2).astype(np.float32)
        p2h, p2m, p2l = _split3(p2)
        ones = np.ones((NTILES, TP), np.float32)
        RR = np.stack([
            pyh, pym, pyh, pym, pyl, pyh,
            pxh, pxm, pxh, pxm, pxl, pxh,
            p2h, p2m, p2l,
            ones, ones, ones,
        ], axis=1).astype(np.float32)                  # [NTILES, NR, TP]

        tsm = (tau * sm).astype(np.float32)
        W3 = _bf16(np.stack([
            np.broadcast_to(tsm[None, :], (NTILES, N)),
            xt * tsm[None, :],
            yt * tsm[None, :],
        ], axis=1).astype(np.float32))                 # [NTILES, 3, N]

        for i in range(NCORES):
            tl = slice(i * TPC, (i + 1) * TPC)
            # rr: rows replicated into the 4 row-tile partition groups
            for gk in range(NK):
                rr_all[i, b, :, 32 * gk : 32 * gk + NR, :] = RR[tl]
                # w1: partition group gk rows 0..NR-1 = weight rows of block gk
                w1_all[i, b, :, 32 * gk : 32 * gk + NR, :] = (
                    W1[tl][:, :, gk * 128 : (gk + 1) * 128]
                )
            # w3: [128, NK*3] with col = k*3 + r ; element [j, k*3+r] = W3[T, r, k*128+j]
            w3c = W3[tl].reshape(TPC, 3, NK, 128).transpose(0, 3, 2, 1)  # [TPC,128,NK,3]
            w3_all[i, b] = w3c.reshape(TPC, 128, NK * 3)
            ptsf_all[i, b, 0] = pyt[tl].reshape(-1)
            ptsf_all[i, b, 1] = pxt[tl].reshape(-1)

    in_maps = []
    for i in range(NCORES):
        in_maps.append({
            "rr": rr_all[i].astype(mbf),
            "w1": w1_all[i].astype(mbf),
            "w3": w3_all[i].astype(mbf),
            "c2": c2,
            "ptsf": np.ascontiguousarray(ptsf_all[i].reshape(B, 2, 128, PPB // 128)),
        })
    return in_maps, perms


def _assemble(results, perms):
    out = np.zeros((B, P, 2), dtype=np.float32)
    for b in range(B):
        vals = np.concatenate(
            [np.asarray(results[i]["out"])[b].reshape(2, PPB) for i in range(NCORES)],
            axis=1,
        )  # [2, P] in sorted order
        out[b, perms[b], 0] = vals[0]
        out[b, perms[b], 1] = vals[1]
    return out.reshape(B, H, W, 2)


def _run(vortex_feature, points, trace=False):
    _, _, _, run_bass_kernel_spmd, _b = _bass_modules()
    if "nc" not in _cache:
        _cache["nc"] = _build_nc()
    in_maps, perms = _prep_inputs(vortex_feature, points)
    res = run_bass_kernel_spmd(
        _cache["nc"], in_maps, list(range(NCORES)), trace=trace
    )
    return _assemble(res.results, perms), res


def kernel(vortex_feature, points):
    out, _ = _run(vortex_feature, points, trace=False)
    return out
